# Initial kernel scaffold
#
"""Trainium2 Bass kernel for autoregressive LSTM categorical sampling.

Strategy:
- Data parallel: batch 16384 split as 2048 per core across 8 cores.
- jax.random.categorical(k, logits) == argmax(logits + gumbel(k)): the gumbel
  noise is independent of logits, so it is precomputed bit-exactly on host CPU
  (threefry) and streamed to the device.
- emb[tok] @ w_ih.T == (embed @ w_ih.T)[tok]: precompute the 64 x 2048 table
  E = embed @ w_ih.T + b_ih + b_hh on host; on device the lookup is a one-hot
  matmul (K=64) accumulated into the same PSUM as the h @ w_hh.T matmuls.
- The BOS step (h=c=0) produces identical state for every batch row; it is
  computed on host with CPU jax (replicating the reference exactly) and
  uploaded as the initial (h, c, logits).
- Device layout: hidden on partitions, batch on the free dim for the LSTM;
  batch on partitions, alphabet on the free dim for sampling (native argmax
  via max/max_index and free-dim reductions). The bridge between the layouts
  is a PE transpose of the one-hot matrix.
- All matmuls in fp32 (4 cyc/row) to track the fp32 CPU reference as closely
  as possible: the sampled-token feedback makes logit errors chaotic.
"""

import os
import numpy as np

L = 256          # decode steps
A = 64           # alphabet
H = 512          # hidden
BATCH = 16384
NCORES = 8
BC = BATCH // NCORES   # batch per core = 2048
NT = BC // 128         # batch tiles of 128 = 16
NG = 4                 # groups of 512 batch columns
GB = BC // NG          # 512
KC = H // 128          # 4 hidden chunks
FourH = 4 * H          # 2048


def _build_module(steps, loop_mode):
    import sys
    if "/opt/trn_rl_repo" not in sys.path:
        sys.path.insert(0, "/opt/trn_rl_repo")
    import concourse.bass as bass
    import concourse.bacc as bacc
    import concourse.tile as tile
    import concourse.mybir as mybir

    F32 = mybir.dt.float32
    I32 = mybir.dt.int32
    U32 = mybir.dt.uint32
    AFT = mybir.ActivationFunctionType
    ALU = mybir.AluOpType
    ds = bass.ds

    nc = bacc.Bacc("TRN2", target_bir_lowering=False, debug=False)

    g_d = nc.dram_tensor("g", [steps, BC, A], F32, kind="ExternalInput")
    h0_d = nc.dram_tensor("h0t", [128, KC * BC], F32, kind="ExternalInput")
    c0_d = nc.dram_tensor("c0t", [128, KC * BC], F32, kind="ExternalInput")
    lg0_d = nc.dram_tensor("lg0", [128, NT * A], F32, kind="ExternalInput")
    wt_d = nc.dram_tensor("wt", [128, KC * FourH], F32, kind="ExternalInput")
    et_d = nc.dram_tensor("et", [A, FourH], F32, kind="ExternalInput")
    hwt_d = nc.dram_tensor("hwt", [128, KC * A], F32, kind="ExternalInput")
    hbr_d = nc.dram_tensor("hbr", [128, 4 * A], F32, kind="ExternalInput")
    iota_d = nc.dram_tensor("iotaf", [128, A], F32, kind="ExternalInput")
    id_d = nc.dram_tensor("ident", [128, 128], F32, kind="ExternalInput")
    xr_d = nc.dram_tensor("xr", [steps, NT * 128], I32, kind="ExternalOutput")
    lp_d = nc.dram_tensor("lp", [128, NT], F32, kind="ExternalOutput")

    with tile.TileContext(nc) as tc:
        with (
            tc.tile_pool(name="const", bufs=1) as cpool,
            tc.tile_pool(name="state", bufs=1) as spool,
            tc.tile_pool(name="work", bufs=2) as work,
            tc.tile_pool(name="nl", bufs=2) as nlpool,
            tc.tile_pool(name="gpsum", bufs=4, space="PSUM") as gps_pool,
            tc.tile_pool(name="hpsum", bufs=2, space="PSUM") as hd_pool,
            tc.tile_pool(name="tpsum", bufs=1, space="PSUM") as tp_pool,
            tc.tile_pool(name="xpsum", bufs=1, space="PSUM") as xp_pool,
        ):
            # --- constants / weights ---
            wt_sb = cpool.tile([128, KC * FourH], F32, name="wt_sb")
            et_sb = cpool.tile([A, FourH], F32, name="et_sb")
            hwt_sb = cpool.tile([128, KC * A], F32, name="hwt_sb")
            hbr_sb = cpool.tile([128, 4 * A], F32, name="hbr_sb")
            iota_sb = cpool.tile([128, A], F32, name="iota_sb")
            id_sb = cpool.tile([128, 128], F32, name="id_sb")
            nc.sync.dma_start(wt_sb[:], wt_d[:])
            nc.sync.dma_start(et_sb[:], et_d[:])
            nc.sync.dma_start(hwt_sb[:], hwt_d[:])
            nc.sync.dma_start(hbr_sb[:], hbr_d[:])
            nc.sync.dma_start(iota_sb[:], iota_d[:])
            nc.sync.dma_start(id_sb[:], id_d[:])

            # --- state (ping-pong h, in-place c, ping-pong logits) ---
            hA = spool.tile([128, KC * BC], F32, name="hA")
            hB = spool.tile([128, KC * BC], F32, name="hB")
            cT = spool.tile([128, KC * BC], F32, name="cT")
            lgA = spool.tile([128, NT * A], F32, name="lgA")
            lgB = spool.tile([128, NT * A], F32, name="lgB")
            lp_sb = spool.tile([128, NT], F32, name="lp_sb")
            gA = spool.tile([128, NT * A], F32, name="gA")
            gB = spool.tile([128, NT * A], F32, name="gB")
            nc.sync.dma_start(hA[:], h0_d[:])
            nc.sync.dma_start(cT[:], c0_d[:])
            nc.sync.dma_start(lgA[:], lg0_d[:])
            nc.vector.memset(lp_sb[:], 0.0)

            def dma_g(dst, t_ap, skip_oob=False):
                src = g_d[t_ap, :, :] if isinstance(t_ap, int) else g_d[ds(t_ap, 1), :, :]
                src = src.rearrange("o (j p) a -> p (o j a)", p=128)
                kw = {}
                if skip_oob:
                    kw["bounds_check"] = "skip_entire_dma"
                nc.sync.dma_start(dst[:], src, **kw)

            def emit_step(t_ap, hT_cur, hT_nxt, lg_cur, lg_nxt, Gt):
                # ---- phase 1: sampling for all 16 tiles ----
                xcol = work.tile([128, NT], F32, tag="xcol", name="xcol")
                ohbs = []
                for j in range(NT):
                    lgs = lg_cur[:, A * j:A * j + A]
                    V = work.tile([128, A], F32, tag="V", name="V", bufs=3)
                    nc.vector.tensor_add(V[:], lgs, Gt[:, A * j:A * j + A])
                    mx8 = work.tile([128, 8], F32, tag="mx8", name="mx8", bufs=3)
                    nc.vector.max(mx8[:], V[:])
                    idx8 = work.tile([128, 8], U32, tag="idx8", name="idx8", bufs=3)
                    nc.vector.max_index(idx8[:], mx8[:], V[:])
                    nc.vector.tensor_copy(xcol[:, j:j + 1], idx8[:, 0:1])
                    ohb = work.tile([128, A], F32, tag=f"ohb{j % 8}",
                                    name=f"ohb{j % 8}", bufs=2)
                    nc.vector.tensor_scalar(ohb[:], iota_sb[:], xcol[:, j:j + 1],
                                            None, op0=ALU.is_equal)
                    scr = work.tile([128, A], F32, tag="scr", name="scr", bufs=3)
                    ltok = work.tile([128, 1], F32, tag="ltok", name="ltok", bufs=3)
                    nc.vector.tensor_tensor_reduce(
                        scr[:], lgs, ohb[:], 1.0, 0.0,
                        ALU.mult, ALU.add, accum_out=ltok[:])
                    ex = work.tile([128, A], F32, tag="ex", name="ex", bufs=3)
                    S = work.tile([128, 1], F32, tag="S", name="S", bufs=3)
                    nc.scalar.activation(ex[:], lgs, AFT.Exp, accum_out=S[:])
                    lnS = work.tile([128, 1], F32, tag="lnS", name="lnS", bufs=3)
                    nc.scalar.activation(lnS[:], S[:], AFT.Ln)
                    dl = work.tile([128, 1], F32, tag="dl", name="dl", bufs=3)
                    nc.vector.tensor_sub(dl[:], ltok[:], lnS[:])
                    nc.vector.tensor_add(lp_sb[:, j:j + 1], lp_sb[:, j:j + 1], dl[:])
                    ohbs.append(ohb)

                # ---- x output: transpose token column block, DMA out ----
                x_ps = xp_pool.tile([NT, 128], F32, tag="x_ps", name="x_ps")
                nc.tensor.transpose(x_ps[:], xcol[:], id_sb[:])
                xrow = work.tile([NT, 128], I32, tag="xrow", name="xrow")
                nc.vector.tensor_copy(xrow[:], x_ps[:])
                if isinstance(t_ap, int):
                    nc.sync.dma_start(xr_d[t_ap, :].rearrange("(j p) -> j p", p=128),
                                      xrow[:])
                else:
                    nc.sync.dma_start(
                        xr_d[ds(t_ap, 1), :].rearrange("o (j p) -> j (o p)", p=128),
                        xrow[:])

                # ---- per group: onehot transpose, gates, state, head ----
                for g in range(NG):
                    oht_ps = tp_pool.tile([A, GB], F32, tag="oht_ps", name="oht_ps")
                    for j4 in range(4):
                        nc.tensor.transpose(oht_ps[:, 128 * j4:128 * j4 + 128],
                                            ohbs[4 * g + j4][:], id_sb[:])
                    ohT = work.tile([A, GB], F32, tag="ohT", name="ohT")
                    nc.vector.tensor_copy(ohT[:], oht_ps[:])

                    gps = {}
                    for hc in range(KC):
                        for gt in range(4):
                            m = 4 * gt + hc
                            p = gps_pool.tile([128, GB], F32, tag=f"gps{(m % 4)}",
                                              name=f"gps{m % 4}", bufs=4)
                            nc.tensor.matmul(p[:], et_sb[:, 128 * m:128 * m + 128],
                                             ohT[:], start=True, stop=False)
                            for c in range(KC):
                                nc.tensor.matmul(
                                    p[:],
                                    wt_sb[:, FourH * c + 128 * m:FourH * c + 128 * m + 128],
                                    hT_cur[:, BC * c + GB * g:BC * c + GB * g + GB],
                                    start=False, stop=(c == KC - 1))
                            gps[gt] = p
                        # nonlinearity + state update for hidden chunk hc
                        cs = cT[:, BC * hc + GB * g:BC * hc + GB * g + GB]
                        hs = hT_nxt[:, BC * hc + GB * g:BC * hc + GB * g + GB]
                        ti = nlpool.tile([128, GB], F32, tag="ti", name="ti")
                        tf = nlpool.tile([128, GB], F32, tag="tf", name="tf")
                        tg = nlpool.tile([128, GB], F32, tag="tg", name="tg")
                        to = nlpool.tile([128, GB], F32, tag="to", name="to")
                        nc.scalar.activation(ti[:], gps[0][:], AFT.Sigmoid)
                        nc.scalar.activation(tf[:], gps[1][:], AFT.Sigmoid)
                        nc.scalar.activation(tg[:], gps[2][:], AFT.Tanh)
                        nc.scalar.activation(to[:], gps[3][:], AFT.Sigmoid)
                        nc.vector.tensor_mul(ti[:], ti[:], tg[:])
                        nc.vector.tensor_mul(tf[:], tf[:], cs)
                        nc.vector.tensor_add(cs, ti[:], tf[:])
                        nc.scalar.activation(tg[:], cs, AFT.Tanh)
                        nc.vector.tensor_mul(hs, to[:], tg[:])

                    # head for this group
                    hd_ps = hd_pool.tile([128, 4 * A], F32, tag="hd_ps", name="hd_ps")
                    for j4 in range(4):
                        for c in range(KC):
                            nc.tensor.matmul(
                                hd_ps[:, A * j4:A * j4 + A],
                                hT_nxt[:, BC * c + GB * g + 128 * j4:
                                       BC * c + GB * g + 128 * j4 + 128],
                                hwt_sb[:, A * c:A * c + A],
                                start=(c == 0), stop=(c == KC - 1))
                    nc.vector.tensor_add(lg_nxt[:, 4 * A * g:4 * A * g + 4 * A],
                                         hd_ps[:], hbr_sb[:])

            if loop_mode == "unroll":
                for t in range(steps):
                    cur, nxt = (hA, hB) if t % 2 == 0 else (hB, hA)
                    lgc, lgn = (lgA, lgB) if t % 2 == 0 else (lgB, lgA)
                    Gt = gA if t % 2 == 0 else gB
                    dma_g(Gt, t)
                    emit_step(t, cur, nxt, lgc, lgn, Gt)
            else:
                PE = mybir.EngineType.PE
                ACT = mybir.EngineType.Activation
                DVE = mybir.EngineType.DVE
                SP = mybir.EngineType.SP
                POOL = mybir.EngineType.Pool
                dma_g(gA, 0)
                with tc.For_i(0, steps, 2,
                              hint_engines=(PE, ACT, DVE, SP, POOL)) as i:
                    dma_g(gB, i + 1)
                    emit_step(i, hA, hB, lgA, lgB, gA)
                    dma_g(gA, i + 2, skip_oob=True)
                    emit_step(i + 1, hB, hA, lgB, lgA, gB)

            nc.sync.dma_start(lp_d[:], lp_sb[:])

    nc.compile()
    return nc


def _host_precompute(embed, w_ih, w_hh, b_ih, b_hh, head_w, head_b, steps):
    """CPU-jax computations that must bit-match the reference: gumbel noise
    and the BOS step."""
    import jax
    import jax.numpy as jnp
    cpu = jax.devices("cpu")[0]
    with jax.default_device(cpu):
        put = lambda v: jax.device_put(np.asarray(v), cpu)
        embed_j = put(embed)
        w_ih_j = put(w_ih)
        w_hh_j = put(w_hh)
        b_ih_j = put(b_ih)
        b_hh_j = put(b_hh)
        head_w_j = put(head_w)
        head_b_j = put(head_b)

        # BOS step exactly as the reference (h0 = c0 = 0, full batch)
        dt = embed_j.dtype
        h = jnp.zeros((BATCH, H), dtype=dt)
        c = jnp.zeros((BATCH, H), dtype=dt)
        emb_bos = jnp.broadcast_to(embed_j[A], (BATCH, H))
        gates = emb_bos @ w_ih_j.T + b_ih_j + h @ w_hh_j.T + b_hh_j
        i_, f_, g_, o_ = jnp.split(gates, 4, axis=-1)
        c1 = jax.nn.sigmoid(f_) * c + jax.nn.sigmoid(i_) * jnp.tanh(g_)
        h1 = jax.nn.sigmoid(o_) * jnp.tanh(c1)
        logits1 = h1 @ head_w_j.T + head_b_j
        h1 = np.asarray(h1)
        c1 = np.asarray(c1)
        logits1 = np.asarray(logits1)

        # gumbel noise, bit-exact threefry on CPU
        keys = jax.random.split(jax.random.key(42), L)
        gum = np.empty((steps, BATCH, A), dtype=np.float32)
        chunk = 16
        for s in range(0, steps, chunk):
            e = min(steps - s, chunk)
            ks = keys[s:s + e]
            arr = jax.vmap(lambda k: jax.random.gumbel(k, (BATCH, A), dt))(ks)
            gum[s:s + e] = np.asarray(arr)

        # E table: embed @ w_ih.T + b_ih + b_hh for tokens 0..63
        E = np.asarray(embed_j[:A] @ w_ih_j.T + b_ih_j + b_hh_j)

    return h1, c1, logits1, gum, E


def _arrange_core_inputs(h1, c1, logits1, gum, E, w_hh, head_w, head_b, core, steps):
    """Build the per-core input map in device layouts."""
    sl = slice(BC * core, BC * (core + 1))
    h1c = h1[sl]          # (2048, 512)
    c1c = c1[sl]
    lg1c = logits1[sl]    # (2048, 64)
    g_c = gum[:, sl, :]   # (steps, 2048, 64)

    # hT layout: [p, BC*c + b] = h[b, 128c + p]
    h0t = np.ascontiguousarray(
        h1c.reshape(BC, KC, 128).transpose(2, 1, 0).reshape(128, KC * BC))
    c0t = np.ascontiguousarray(
        c1c.reshape(BC, KC, 128).transpose(2, 1, 0).reshape(128, KC * BC))
    # logits layout: [p, A*j + a] = logits[128j + p, a]
    lg0 = np.ascontiguousarray(
        lg1c.reshape(NT, 128, A).transpose(1, 0, 2).reshape(128, NT * A))
    return {
        "g": np.ascontiguousarray(g_c),
        "h0t": h0t,
        "c0t": c0t,
        "lg0": lg0,
    }


def _shared_inputs(E, w_hh, head_w, head_b):
    # wt layout: [p, FourH*c + q] = w_hh[q, 128c + p]
    wt = np.ascontiguousarray(
        w_hh.reshape(FourH, KC, 128).transpose(2, 1, 0).reshape(128, KC * FourH))
    # hwt layout: [p, A*c + a] = head_w[a, 128c + p]
    hwt = np.ascontiguousarray(
        head_w.reshape(A, KC, 128).transpose(2, 1, 0).reshape(128, KC * A))
    hbr = np.ascontiguousarray(np.tile(head_b[None, :], (128, 4)))
    iota = np.ascontiguousarray(
        np.tile(np.arange(A, dtype=np.float32)[None, :], (128, 1)))
    ident = np.eye(128, dtype=np.float32)
    return {
        "wt": wt.astype(np.float32),
        "et": np.ascontiguousarray(E.astype(np.float32)),
        "hwt": hwt.astype(np.float32),
        "hbr": hbr.astype(np.float32),
        "iotaf": iota,
        "ident": ident,
    }


_module_cache = {}


def _get_module(steps, loop_mode):
    key = (steps, loop_mode)
    if key not in _module_cache:
        _module_cache[key] = _build_module(steps, loop_mode)
    return _module_cache[key]


def run_device(inputs, steps=L, loop_mode="for_i", trace=False):
    """Run the device portion; returns (x, logp) full-batch plus timing info."""
    import sys
    if "/opt/trn_rl_repo" not in sys.path:
        sys.path.insert(0, "/opt/trn_rl_repo")
    from concourse.bass_utils import run_bass_kernel_spmd

    embed = np.asarray(inputs["embed"], dtype=np.float32)
    w_ih = np.asarray(inputs["w_ih"], dtype=np.float32)
    w_hh = np.asarray(inputs["w_hh"], dtype=np.float32)
    b_ih = np.asarray(inputs["b_ih"], dtype=np.float32)
    b_hh = np.asarray(inputs["b_hh"], dtype=np.float32)
    head_w = np.asarray(inputs["head_w"], dtype=np.float32)
    head_b = np.asarray(inputs["head_b"], dtype=np.float32)

    h1, c1, logits1, gum, E = _host_precompute(
        embed, w_ih, w_hh, b_ih, b_hh, head_w, head_b, steps)
    shared = _shared_inputs(E, w_hh, head_w, head_b)

    in_maps = []
    for core in range(NCORES):
        m = _arrange_core_inputs(h1, c1, logits1, gum, E, w_hh, head_w, head_b,
                                 core, steps)
        m.update(shared)
        in_maps.append(m)

    nc = _get_module(steps, loop_mode)
    res = run_bass_kernel_spmd(nc, in_maps, list(range(NCORES)), trace=trace)

    xs = []
    lps = []
    for core in range(NCORES):
        r = res.results[core]
        xr = r["xr"]          # (steps, NT*128) int32; [t, 128j + p] = tok[128j+p]
        lp = r["lp"]          # (128, NT)
        xs.append(np.ascontiguousarray(xr.reshape(steps, BC).T))
        lps.append(np.ascontiguousarray(lp.T.reshape(BC)))
    x = np.concatenate(xs, axis=0).astype(np.int32)
    logp = np.concatenate(lps, axis=0).astype(np.float32)
    return x, logp, res


def kernel(**inputs):
    x, logp, _ = run_device(inputs, steps=L, loop_mode="for_i")
    return x, logp


# revision 12
# speedup vs baseline: 1.9989x; 1.9989x over previous
"""Trainium2 Bass kernel for autoregressive LSTM categorical sampling.

Strategy:
- Data parallel: batch 16384 split as 2048 per core across 8 cores.
- jax.random.categorical(k, logits) == argmax(logits + gumbel(k)): the gumbel
  noise is independent of logits, so it is precomputed bit-exactly on host CPU
  (threefry) and streamed to the device.
- emb[tok] @ w_ih.T == (embed @ w_ih.T)[tok]: precompute the 64 x 2048 table
  E = embed @ w_ih.T + b_ih + b_hh on host; on device the lookup is a one-hot
  matmul (K=64) accumulated into the same PSUM as the h @ w_hh.T matmuls.
- The BOS step (h=c=0) produces identical state for every batch row; it is
  computed on host with CPU jax (replicating the reference exactly) and
  uploaded as the initial (h, c, logits).
- Device layout: hidden on partitions, batch on the free dim for the LSTM;
  batch on partitions, alphabet on the free dim for sampling (native argmax
  via max/max_index and free-dim reductions). The bridge between the layouts
  is a PE transpose of the one-hot matrix.
- All matmuls in fp32 (4 cyc/row) to track the fp32 CPU reference as closely
  as possible: the sampled-token feedback makes logit errors chaotic.
"""

import os
import numpy as np

L = 256          # decode steps
A = 64           # alphabet
H = 512          # hidden
BATCH = 16384
NCORES = 8
BC = BATCH // NCORES   # batch per core = 2048
NT = BC // 128         # batch tiles of 128 = 16
NG = 4                 # groups of 512 batch columns
GB = BC // NG          # 512
KC = H // 128          # 4 hidden chunks
FourH = 4 * H          # 2048


def _build_module(steps, loop_mode):
    import sys
    if "/opt/trn_rl_repo" not in sys.path:
        sys.path.insert(0, "/opt/trn_rl_repo")
    import concourse.bass as bass
    import concourse.bacc as bacc
    import concourse.tile as tile
    import concourse.mybir as mybir
    from concourse.expressions import smin

    F32 = mybir.dt.float32
    I32 = mybir.dt.int32
    U32 = mybir.dt.uint32
    AFT = mybir.ActivationFunctionType
    ALU = mybir.AluOpType
    ds = bass.ds

    nc = bacc.Bacc("TRN2", target_bir_lowering=False, debug=False)

    g_d = nc.dram_tensor("g", [steps, BC, A], F32, kind="ExternalInput")
    h0_d = nc.dram_tensor("h0t", [128, KC * BC], F32, kind="ExternalInput")
    c0_d = nc.dram_tensor("c0t", [128, KC * BC], F32, kind="ExternalInput")
    lg0_d = nc.dram_tensor("lg0", [128, NT * A], F32, kind="ExternalInput")
    wt_d = nc.dram_tensor("wt", [128, KC * FourH], F32, kind="ExternalInput")
    et_d = nc.dram_tensor("et", [A, FourH], F32, kind="ExternalInput")
    hwt_d = nc.dram_tensor("hwt", [128, KC * A], F32, kind="ExternalInput")
    hbr_d = nc.dram_tensor("hbr", [128, 4 * A], F32, kind="ExternalInput")
    iota_d = nc.dram_tensor("iotaf", [128, A], F32, kind="ExternalInput")
    id_d = nc.dram_tensor("ident", [128, 128], F32, kind="ExternalInput")
    xr_d = nc.dram_tensor("xr", [steps, NT * 128], I32, kind="ExternalOutput")
    lp_d = nc.dram_tensor("lp", [128, NT], F32, kind="ExternalOutput")

    with tile.TileContext(nc) as tc:
        with (
            tc.tile_pool(name="const", bufs=1) as cpool,
            tc.tile_pool(name="state", bufs=1) as spool,
            tc.tile_pool(name="work", bufs=2) as work,
            tc.tile_pool(name="nl", bufs=2) as nlpool,
            tc.tile_pool(name="gpsum", bufs=4, space="PSUM") as gps_pool,
            tc.tile_pool(name="hpsum", bufs=2, space="PSUM") as hd_pool,
            tc.tile_pool(name="tpsum", bufs=1, space="PSUM") as tp_pool,
            tc.tile_pool(name="xpsum", bufs=1, space="PSUM") as xp_pool,
        ):
            # --- constants / weights ---
            wt_sb = cpool.tile([128, KC * FourH], F32, name="wt_sb")
            et_sb = cpool.tile([A, FourH], F32, name="et_sb")
            hwt_sb = cpool.tile([128, KC * A], F32, name="hwt_sb")
            hbr_sb = cpool.tile([128, 4 * A], F32, name="hbr_sb")
            iota_sb = cpool.tile([128, A], F32, name="iota_sb")
            id_sb = cpool.tile([128, 128], F32, name="id_sb")
            nc.sync.dma_start(wt_sb[:], wt_d[:])
            nc.sync.dma_start(et_sb[:], et_d[:])
            nc.sync.dma_start(hwt_sb[:], hwt_d[:])
            nc.sync.dma_start(hbr_sb[:], hbr_d[:])
            nc.sync.dma_start(iota_sb[:], iota_d[:])
            nc.sync.dma_start(id_sb[:], id_d[:])

            # --- state (ping-pong h, in-place c, ping-pong logits) ---
            hA = spool.tile([128, KC * BC], F32, name="hA")
            hB = spool.tile([128, KC * BC], F32, name="hB")
            cT = spool.tile([128, KC * BC], F32, name="cT")
            lgA = spool.tile([128, NT * A], F32, name="lgA")
            lgB = spool.tile([128, NT * A], F32, name="lgB")
            lp_sb = spool.tile([128, NT], F32, name="lp_sb")
            gA = spool.tile([128, NT * A], F32, name="gA")
            gB = spool.tile([128, NT * A], F32, name="gB")
            nc.sync.dma_start(hA[:], h0_d[:])
            nc.sync.dma_start(cT[:], c0_d[:])
            nc.sync.dma_start(lgA[:], lg0_d[:])
            nc.vector.memset(lp_sb[:], 0.0)

            def dma_g(dst, t_ap, skip_oob=False):
                if isinstance(t_ap, int):
                    src = g_d[t_ap, :, :].rearrange("(j p) a -> p j a", p=128)
                else:
                    src = g_d[ds(t_ap, 1), :, :].rearrange(
                        "o (j p) a -> p o j a", p=128)
                    src = src.rearrange("p o j a -> p (o j) a")
                kw = {}
                if skip_oob:
                    kw["bounds_check"] = "skip_entire_dma"
                dstv = dst[:].rearrange("p (j a) -> p j a", a=A)
                nc.sync.dma_start(dstv, src, **kw)

            def emit_step(t_ap, hT_cur, hT_nxt, lg_cur, lg_nxt, Gt):
                # ---- phase 1: sampling for all 16 tiles ----
                xcol = work.tile([128, NT], F32, tag="xcol", name="xcol")
                ohbs = []
                for j in range(NT):
                    lgs = lg_cur[:, A * j:A * j + A]
                    V = work.tile([128, A], F32, tag="V", name="V", bufs=3)
                    nc.vector.tensor_add(V[:], lgs, Gt[:, A * j:A * j + A])
                    mx8 = work.tile([128, 8], F32, tag="mx8", name="mx8", bufs=3)
                    nc.vector.max(mx8[:], V[:])
                    idx8 = work.tile([128, 8], U32, tag="idx8", name="idx8", bufs=3)
                    nc.vector.max_index(idx8[:], mx8[:], V[:])
                    nc.vector.tensor_copy(xcol[:, j:j + 1], idx8[:, 0:1])
                    ohb = work.tile([128, A], F32, tag=f"ohb{j % 8}",
                                    name=f"ohb{j % 8}", bufs=2)
                    nc.vector.tensor_scalar(ohb[:], iota_sb[:], xcol[:, j:j + 1],
                                            None, op0=ALU.is_equal)
                    scr = work.tile([128, A], F32, tag="scr", name="scr", bufs=3)
                    ltok = work.tile([128, 1], F32, tag="ltok", name="ltok", bufs=3)
                    nc.vector.tensor_mul(scr[:], lgs, ohb[:])
                    nc.vector.tensor_reduce(ltok[:], scr[:], mybir.AxisListType.X,
                                            ALU.add)
                    ex = work.tile([128, A], F32, tag="ex", name="ex", bufs=3)
                    S = work.tile([128, 1], F32, tag="S", name="S", bufs=3)
                    nc.scalar.activation(ex[:], lgs, AFT.Exp, accum_out=S[:])
                    lnS = work.tile([128, 1], F32, tag="lnS", name="lnS", bufs=3)
                    nc.scalar.activation(lnS[:], S[:], AFT.Ln)
                    dl = work.tile([128, 1], F32, tag="dl", name="dl", bufs=3)
                    nc.vector.tensor_sub(dl[:], ltok[:], lnS[:])
                    nc.vector.tensor_add(lp_sb[:, j:j + 1], lp_sb[:, j:j + 1], dl[:])
                    ohbs.append(ohb)

                # ---- x output: transpose token column block, DMA out ----
                x_ps = xp_pool.tile([NT, 128], F32, tag="x_ps", name="x_ps")
                nc.tensor.transpose(x_ps[:], xcol[:], id_sb[:])
                xrow = work.tile([NT, 128], I32, tag="xrow", name="xrow")
                nc.vector.tensor_copy(xrow[:], x_ps[:])
                if isinstance(t_ap, int):
                    nc.sync.dma_start(xr_d[t_ap, :].rearrange("(j p) -> j p", p=128),
                                      xrow[:])
                else:
                    dst = xr_d[ds(t_ap, 1), :].rearrange("o (j p) -> o j p", p=128)
                    nc.sync.dma_start(dst.rearrange("o j p -> (o j) p"), xrow[:])

                # ---- per group: onehot transpose, gates, state, head ----
                for g in range(NG):
                    oht_ps = tp_pool.tile([A, GB], F32, tag="oht_ps", name="oht_ps")
                    for j4 in range(4):
                        nc.tensor.transpose(oht_ps[:, 128 * j4:128 * j4 + 128],
                                            ohbs[4 * g + j4][:], id_sb[:])
                    ohT = work.tile([A, GB], F32, tag="ohT", name="ohT")
                    nc.vector.tensor_copy(ohT[:], oht_ps[:])

                    gps = {}
                    for hc in range(KC):
                        for gt in range(4):
                            m = 4 * gt + hc
                            p = gps_pool.tile([128, GB], F32, tag="gps",
                                              name="gps", bufs=4)
                            nc.tensor.matmul(p[:], et_sb[:, 128 * m:128 * m + 128],
                                             ohT[:], start=True, stop=False)
                            for c in range(KC):
                                nc.tensor.matmul(
                                    p[:],
                                    wt_sb[:, FourH * c + 128 * m:FourH * c + 128 * m + 128],
                                    hT_cur[:, BC * c + GB * g:BC * c + GB * g + GB],
                                    start=False, stop=(c == KC - 1))
                            gps[gt] = p
                        # nonlinearity + state update for hidden chunk hc
                        cs = cT[:, BC * hc + GB * g:BC * hc + GB * g + GB]
                        hs = hT_nxt[:, BC * hc + GB * g:BC * hc + GB * g + GB]
                        ti = nlpool.tile([128, GB], F32, tag="ti", name="ti")
                        tf = nlpool.tile([128, GB], F32, tag="tf", name="tf")
                        tg = nlpool.tile([128, GB], F32, tag="tg", name="tg")
                        to = nlpool.tile([128, GB], F32, tag="to", name="to")
                        nc.scalar.activation(ti[:], gps[0][:], AFT.Sigmoid)
                        nc.scalar.activation(tf[:], gps[1][:], AFT.Sigmoid)
                        nc.scalar.activation(tg[:], gps[2][:], AFT.Tanh)
                        nc.scalar.activation(to[:], gps[3][:], AFT.Sigmoid)
                        nc.vector.tensor_mul(ti[:], ti[:], tg[:])
                        nc.vector.tensor_mul(tf[:], tf[:], cs)
                        nc.vector.tensor_add(cs, ti[:], tf[:])
                        nc.scalar.activation(tg[:], cs, AFT.Tanh)
                        nc.vector.tensor_mul(hs, to[:], tg[:])

                    # head for this group
                    hd_ps = hd_pool.tile([128, 4 * A], F32, tag="hd_ps", name="hd_ps")
                    for j4 in range(4):
                        for c in range(KC):
                            nc.tensor.matmul(
                                hd_ps[:, A * j4:A * j4 + A],
                                hT_nxt[:, BC * c + GB * g + 128 * j4:
                                       BC * c + GB * g + 128 * j4 + 128],
                                hwt_sb[:, A * c:A * c + A],
                                start=(c == 0), stop=(c == KC - 1))
                    nc.vector.tensor_add(lg_nxt[:, 4 * A * g:4 * A * g + 4 * A],
                                         hd_ps[:], hbr_sb[:])

            if loop_mode == "unroll":
                for t in range(steps):
                    cur, nxt = (hA, hB) if t % 2 == 0 else (hB, hA)
                    lgc, lgn = (lgA, lgB) if t % 2 == 0 else (lgB, lgA)
                    Gt = gA if t % 2 == 0 else gB
                    dma_g(Gt, t)
                    emit_step(t, cur, nxt, lgc, lgn, Gt)
            else:
                PE = mybir.EngineType.PE
                ACT = mybir.EngineType.Activation
                DVE = mybir.EngineType.DVE
                SP = mybir.EngineType.SP
                POOL = mybir.EngineType.Pool
                dma_g(gA, 0)
                with tc.For_i(0, steps, 2,
                              hint_engines=(PE, ACT, DVE, SP, POOL)) as i:
                    dma_g(gB, i + 1)
                    emit_step(i, hA, hB, lgA, lgB, gA)
                    dma_g(gA, smin(i + 2, steps - 1))
                    emit_step(i + 1, hB, hA, lgB, lgA, gB)

            nc.sync.dma_start(lp_d[:], lp_sb[:])

    nc.compile()
    return nc


def _host_precompute(embed, w_ih, w_hh, b_ih, b_hh, head_w, head_b, steps):
    """CPU-jax computations that must bit-match the reference: gumbel noise
    and the BOS step."""
    import jax
    import jax.numpy as jnp
    cpu = jax.devices("cpu")[0]
    with jax.default_device(cpu):
        put = lambda v: jax.device_put(np.asarray(v), cpu)
        embed_j = put(embed)
        w_ih_j = put(w_ih)
        w_hh_j = put(w_hh)
        b_ih_j = put(b_ih)
        b_hh_j = put(b_hh)
        head_w_j = put(head_w)
        head_b_j = put(head_b)

        # BOS step exactly as the reference (h0 = c0 = 0, full batch)
        dt = embed_j.dtype
        h = jnp.zeros((BATCH, H), dtype=dt)
        c = jnp.zeros((BATCH, H), dtype=dt)
        emb_bos = jnp.broadcast_to(embed_j[A], (BATCH, H))
        gates = emb_bos @ w_ih_j.T + b_ih_j + h @ w_hh_j.T + b_hh_j
        i_, f_, g_, o_ = jnp.split(gates, 4, axis=-1)
        c1 = jax.nn.sigmoid(f_) * c + jax.nn.sigmoid(i_) * jnp.tanh(g_)
        h1 = jax.nn.sigmoid(o_) * jnp.tanh(c1)
        logits1 = h1 @ head_w_j.T + head_b_j
        h1 = np.asarray(h1)
        c1 = np.asarray(c1)
        logits1 = np.asarray(logits1)

        # gumbel noise, bit-exact threefry on CPU
        keys = jax.random.split(jax.random.key(42), L)
        gum = np.empty((steps, BATCH, A), dtype=np.float32)
        for t in range(steps):
            gum[t] = np.asarray(jax.random.gumbel(keys[t], (BATCH, A), dt))

        # E table: embed @ w_ih.T + b_ih + b_hh for tokens 0..63
        E = np.asarray(embed_j[:A] @ w_ih_j.T + b_ih_j + b_hh_j)

    return h1, c1, logits1, gum, E


def _arrange_core_inputs(h1, c1, logits1, gum, E, w_hh, head_w, head_b, core, steps):
    """Build the per-core input map in device layouts."""
    sl = slice(BC * core, BC * (core + 1))
    h1c = h1[sl]          # (2048, 512)
    c1c = c1[sl]
    lg1c = logits1[sl]    # (2048, 64)
    g_c = gum[:, sl, :]   # (steps, 2048, 64)

    # hT layout: [p, BC*c + b] = h[b, 128c + p]
    h0t = np.ascontiguousarray(
        h1c.reshape(BC, KC, 128).transpose(2, 1, 0).reshape(128, KC * BC))
    c0t = np.ascontiguousarray(
        c1c.reshape(BC, KC, 128).transpose(2, 1, 0).reshape(128, KC * BC))
    # logits layout: [p, A*j + a] = logits[128j + p, a]
    lg0 = np.ascontiguousarray(
        lg1c.reshape(NT, 128, A).transpose(1, 0, 2).reshape(128, NT * A))
    return {
        "g": np.ascontiguousarray(g_c),
        "h0t": h0t,
        "c0t": c0t,
        "lg0": lg0,
    }


def _shared_inputs(E, w_hh, head_w, head_b):
    # wt layout: [p, FourH*c + q] = w_hh[q, 128c + p]
    wt = np.ascontiguousarray(
        w_hh.reshape(FourH, KC, 128).transpose(2, 1, 0).reshape(128, KC * FourH))
    # hwt layout: [p, A*c + a] = head_w[a, 128c + p]
    hwt = np.ascontiguousarray(
        head_w.reshape(A, KC, 128).transpose(2, 1, 0).reshape(128, KC * A))
    hbr = np.ascontiguousarray(np.tile(head_b[None, :], (128, 4)))
    iota = np.ascontiguousarray(
        np.tile(np.arange(A, dtype=np.float32)[None, :], (128, 1)))
    ident = np.eye(128, dtype=np.float32)
    return {
        "wt": wt.astype(np.float32),
        "et": np.ascontiguousarray(E.astype(np.float32)),
        "hwt": hwt.astype(np.float32),
        "hbr": hbr.astype(np.float32),
        "iotaf": iota,
        "ident": ident,
    }


_module_cache = {}


def _get_module(steps, loop_mode):
    key = (steps, loop_mode)
    if key not in _module_cache:
        _module_cache[key] = _build_module(steps, loop_mode)
    return _module_cache[key]


def run_device(inputs, steps=L, loop_mode="for_i", trace=False, timers=None):
    """Run the device portion; returns (x, logp) full-batch plus timing info."""
    import sys
    import time as _time
    if "/opt/trn_rl_repo" not in sys.path:
        sys.path.insert(0, "/opt/trn_rl_repo")
    from concourse.bass_utils import run_bass_kernel_spmd

    embed = np.asarray(inputs["embed"], dtype=np.float32)
    w_ih = np.asarray(inputs["w_ih"], dtype=np.float32)
    w_hh = np.asarray(inputs["w_hh"], dtype=np.float32)
    b_ih = np.asarray(inputs["b_ih"], dtype=np.float32)
    b_hh = np.asarray(inputs["b_hh"], dtype=np.float32)
    head_w = np.asarray(inputs["head_w"], dtype=np.float32)
    head_b = np.asarray(inputs["head_b"], dtype=np.float32)

    t0 = _time.time()
    h1, c1, logits1, gum, E = _host_precompute(
        embed, w_ih, w_hh, b_ih, b_hh, head_w, head_b, steps)
    shared = _shared_inputs(E, w_hh, head_w, head_b)

    in_maps = []
    for core in range(NCORES):
        m = _arrange_core_inputs(h1, c1, logits1, gum, E, w_hh, head_w, head_b,
                                 core, steps)
        m.update(shared)
        in_maps.append(m)
    t1 = _time.time()
    nc = _get_module(steps, loop_mode)
    t2 = _time.time()
    res = run_bass_kernel_spmd(nc, in_maps, list(range(NCORES)), trace=trace)
    t3 = _time.time()
    if timers is not None:
        timers["host_prep"] = t1 - t0
        timers["build_compile"] = t2 - t1
        timers["spmd_call"] = t3 - t2
        timers["in_maps"] = in_maps

    xs = []
    lps = []
    for core in range(NCORES):
        r = res.results[core]
        xr = r["xr"]          # (steps, NT*128) int32; [t, 128j + p] = tok[128j+p]
        lp = r["lp"]          # (128, NT)
        xs.append(np.ascontiguousarray(xr.reshape(steps, BC).T))
        lps.append(np.ascontiguousarray(lp.T.reshape(BC)))
    x = np.concatenate(xs, axis=0).astype(np.int32)
    logp = np.concatenate(lps, axis=0).astype(np.float32)
    return x, logp, res


def kernel(**inputs):
    x, logp, _ = run_device(inputs, steps=L, loop_mode="for_i")
    return x, logp
